# revision 1
# baseline (speedup 1.0000x reference)
"""Distributed Trainium2 Bass kernel for GQA attention block (B=2, S=2048, D=4096,
32 Q heads / 8 KV heads, RoPE, causal, output projection).

Sharding: 8 cores = 2 batch groups x 4 ranks. Core c handles batch c//4 and the
512 global rows {4*i + (c%4)} of that batch (strided, so the causal-attention
loop structure is identical on every core -> one SPMD graph). Q/K/V/O
projections are computed locally for those rows; K/V shards are AllGathered
within each 4-core batch group; attention + output projection are local.
No output collective is needed (output rows are disjoint).

Compute dtype: bf16 operands, f32 PSUM accumulation.
"""

import math
import os
import numpy as np
import ml_dtypes

N_CORES = 8
B, S, D = 2, 2048, 4096
NQH, NKVH, HD = 32, 8, 128
GROUP = NQH // NKVH
MLOC = S // 4          # 512 local rows per core
P = 128
KT = D // P            # 32 contraction tiles
BF16 = ml_dtypes.bfloat16

_GRAPH_CACHE = {}


def _build_graph():
    import concourse.bass as bass
    import concourse.mybir as mybir
    import concourse.tile as tile
    from concourse import bacc

    fp32 = mybir.dt.float32
    bf16 = mybir.dt.bfloat16

    nc = bacc.Bacc(None, target_bir_lowering=False, num_devices=N_CORES)

    # ---- I/O -------------------------------------------------------------
    xt = nc.declare_dram_parameter("xt", [D, MLOC], bf16, isOutput=False)
    qw = nc.declare_dram_parameter("qw", [D, D], bf16, isOutput=False)
    kw = nc.declare_dram_parameter("kw", [D, NKVH * HD], bf16, isOutput=False)
    vw = nc.declare_dram_parameter("vw", [D, NKVH * HD], bf16, isOutput=False)
    ow = nc.declare_dram_parameter("ow", [D, D], bf16, isOutput=False)
    qb2 = nc.declare_dram_parameter("qb2", [P, NQH], fp32, isOutput=False)
    kb2 = nc.declare_dram_parameter("kb2", [P, NKVH], fp32, isOutput=False)
    vb2 = nc.declare_dram_parameter("vb2", [P, NKVH], fp32, isOutput=False)
    cosT = nc.declare_dram_parameter("cosT", [P, MLOC], fp32, isOutput=False)
    sinT = nc.declare_dram_parameter("sinT", [P, MLOC], fp32, isOutput=False)
    trimask = nc.declare_dram_parameter("trimask", [4 * P, P], bf16, isOutput=False)
    out = nc.declare_dram_parameter("out", [D, MLOC], fp32, isOutput=True)

    RG = [[0, 1, 2, 3], [4, 5, 6, 7]]
    Exp = mybir.ActivationFunctionType.Exp
    A = mybir.AluOpType

    with tile.TileContext(nc) as tc:
        with (
            tc.tile_pool(name="const", bufs=1) as constp,
            tc.tile_pool(name="big", bufs=1) as bigp,
            tc.tile_pool(name="wstream", bufs=5) as wsp,
            tc.tile_pool(name="stage", bufs=3) as stagep,
            tc.tile_pool(name="rope", bufs=2) as ropep,
            tc.tile_pool(name="dram", bufs=1, space="DRAM") as dramp,
        ):
            # ---- constants -------------------------------------------------
            cos_sb = constp.tile([P, MLOC], fp32, tag="cos")
            sin_sb = constp.tile([P, MLOC], fp32, tag="sin")
            nc.sync.dma_start(cos_sb[:, :], cosT[:, :])
            nc.sync.dma_start(sin_sb[:, :], sinT[:, :])
            qb_sb = constp.tile([P, NQH], fp32, tag="qb")
            kb_sb = constp.tile([P, NKVH], fp32, tag="kb")
            vb_sb = constp.tile([P, NKVH], fp32, tag="vb")
            nc.sync.dma_start(qb_sb[:, :], qb2[:, :])
            nc.sync.dma_start(kb_sb[:, :], kb2[:, :])
            nc.sync.dma_start(vb_sb[:, :], vb2[:, :])
            mask_sb = constp.tile([P, 4, P], bf16, tag="mask")
            for r in range(4):
                nc.sync.dma_start(mask_sb[:, r, :], trimask[r * P:(r + 1) * P, :])
            ones_mat = constp.tile([P, P], bf16, tag="ones_mat")
            nc.vector.memset(ones_mat[:, :], 1.0)

            # ---- big SBUF residents ---------------------------------------
            xt_sb = bigp.tile([P, KT, MLOC], bf16, tag="xt")
            nc.sync.dma_start(
                xt_sb[:, :, :], xt[:, :].rearrange("(t p) m -> p t m", p=P))
            qt_sb = bigp.tile([P, NQH, MLOC], bf16, tag="qt")
            gkt_sb = bigp.tile([P, 4 * NKVH, MLOC], bf16, tag="gkt")
            gv_sb = bigp.tile([P, 4 * (MLOC // P), NKVH * HD], bf16, tag="gv")
            ot_sb = bigp.tile([P, KT, MLOC], bf16, tag="ot")

            # ---- DRAM bounce buffers for collectives ----------------------
            ktb = dramp.tile([NKVH * HD, MLOC], bf16, tag="ktb")
            gktb = dramp.tile([4 * NKVH * HD, MLOC], bf16, tag="gktb")
            vbd = dramp.tile([MLOC, NKVH * HD], bf16, tag="vbd")
            gvbd = dramp.tile([4 * MLOC, NKVH * HD], bf16, tag="gvbd")

            def rope_from_psum(psum, bias_col, dst):
                # psum: [128, MLOC] f32 (feat-major, pair-permuted: rows 0:64 = x0,
                # rows 64:128 = x1).  dst: [128, MLOC] bf16 slice.
                # rope(x+b) = (x+b) * [cos;cos] + swap_halves(x+b) * [-sin;sin]
                h = HD // 2
                tmp = ropep.tile([P, MLOC], fp32, tag="ropeT")
                rx = ropep.tile([P, MLOC], fp32, tag="ropeR")
                nc.vector.tensor_scalar_add(tmp[:, :], psum[:, :], bias_col[:, :])
                nc.sync.dma_start(rx[0:h, :], tmp[h:2 * h, :])
                nc.sync.dma_start(rx[h:2 * h, :], tmp[0:h, :])
                nc.vector.tensor_mul(tmp[:, :], tmp[:, :], cos_sb[:, :])
                nc.vector.tensor_mul(rx[:, :], rx[:, :], sin_sb[:, :])
                nc.vector.tensor_add(dst[:, :], tmp[:, :], rx[:, :])

            # ================= K projection + RoPE + AG ====================
            with tc.tile_pool(name="acc_kv", bufs=1, space="PSUM") as accp:
                kps = [accp.tile([P, MLOC], fp32, tag=f"kacc{i}", name=f"kps{i}") for i in range(NKVH)]
                for k in range(KT):
                    kwt = wsp.tile([P, NKVH * HD], bf16, tag="wt")
                    nc.sync.dma_start(kwt[:, :], kw[k * P:(k + 1) * P, :])
                    for kv in range(NKVH):
                        nc.tensor.matmul(
                            kps[kv][:, :], kwt[:, kv * P:(kv + 1) * P],
                            xt_sb[:, k, :], start=(k == 0), stop=(k == KT - 1))
                for kv in range(NKVH):
                    kt_st = stagep.tile([P, MLOC], bf16, tag="kstage")
                    rope_from_psum(kps[kv], kb_sb[:, kv:kv + 1], kt_st)
                    nc.sync.dma_start(ktb[kv * P:(kv + 1) * P, :], kt_st[:, :])
                nc.gpsimd.collective_compute(
                    "AllGather", A.bypass, replica_groups=RG,
                    ins=[ktb[:, :].opt()], outs=[gktb[:, :].opt()])

                # ================= V projection + AG =======================
                vps = [accp.tile([P, MLOC], fp32, tag=f"kacc{i}", name=f"vps{i}") for i in range(NKVH)]
                for k in range(KT):
                    vwt = wsp.tile([P, NKVH * HD], bf16, tag="wt")
                    nc.sync.dma_start(vwt[:, :], vw[k * P:(k + 1) * P, :])
                    for rt in range(4):
                        for fs in range(2):
                            nc.tensor.matmul(
                                vps[rt * 2 + fs][:, :],
                                xt_sb[:, k, rt * P:(rt + 1) * P],
                                vwt[:, fs * 512:(fs + 1) * 512],
                                start=(k == 0), stop=(k == KT - 1))
                for rt in range(4):
                    for fs in range(2):
                        v_st = stagep.tile([P, 512], bf16, tag="vstage")
                        nc.scalar.copy(v_st[:, :], vps[rt * 2 + fs][:, :])
                        nc.sync.dma_start(
                            vbd[rt * P:(rt + 1) * P, fs * 512:(fs + 1) * 512],
                            v_st[:, :])
                nc.gpsimd.collective_compute(
                    "AllGather", A.bypass, replica_groups=RG,
                    ins=[vbd[:, :].opt()], outs=[gvbd[:, :].opt()])

                # fetch gathered K/V via SWDGE (gpsimd) so the AG-wait cannot
                # head-of-line-block the HWDGE weight streams
                nc.gpsimd.dma_start(
                    gkt_sb[:, :, :], gktb[:, :].rearrange("(t p) m -> p t m", p=P))
                nc.gpsimd.dma_start(
                    gv_sb[:, :, :], gvbd[:, :].rearrange("(t p) m -> p t m", p=P))

            # ================= Q projection + RoPE =========================
            with tc.tile_pool(name="acc_q", bufs=1, space="PSUM") as accq:
                for p in range(4):
                    qps = [accq.tile([P, MLOC], fp32, tag=f"kacc{i}", name=f"qps{i}")
                           for i in range(8)]
                    for k in range(KT):
                        qwt = wsp.tile([P, 1024], bf16, tag="wt")
                        nc.sync.dma_start(
                            qwt[:, :], qw[k * P:(k + 1) * P, p * 1024:(p + 1) * 1024])
                        for hh in range(8):
                            nc.tensor.matmul(
                                qps[hh][:, :], qwt[:, hh * P:(hh + 1) * P],
                                xt_sb[:, k, :], start=(k == 0), stop=(k == KT - 1))
                    for hh in range(8):
                        h = p * 8 + hh
                        rope_from_psum(qps[hh], qb_sb[:, h:h + 1], qt_sb[:, h, :])

            _PH = os.environ.get("KPHASE", "full")
            # ================= attention ===================================
            # qtile t = local q rows [128t, +128) = global rows 4*q + j.
            # key block (r, kb): gkt_sb[:, 8*r + hkv, 128*kb : +128],
            #                    gv_sb[:, 4*r + kb, 128*hkv : +128].
            # causal: kb < t full; kb == t diagonal (mask_sb[:, r, :]).
            # Segments of several key blocks share one wide S psum tile so a
            # single Exp covers them.
            with (
                tc.tile_pool(name="sps", bufs=4, space="PSUM") as spsp,
                tc.tile_pool(name="otps", bufs=2, space="PSUM") as otpsp,
                tc.tile_pool(name="denps", bufs=2, space="PSUM") as denpsp,
                tc.tile_pool(name="attw", bufs=6) as attwp,
                tc.tile_pool(name="atte", bufs=3) as attep,
            ):
                for h in range(NQH if _PH != "kvq" else 0):
                    hkv = h // GROUP
                    for tp in range(2):      # qtile pair (2tp, 2tp+1)
                        t0, t1 = 2 * tp, 2 * tp + 1
                        qs = 2 * tp * P      # local q col offset of the pair
                        otp = otpsp.tile([P, 2 * P], fp32, tag="otp")
                        den = denpsp.tile([P, 2 * P], fp32, tag="den")
                        first = True
                        for r in range(4):
                            # segments: (kb, qoff within pair, width, mask_col)
                            segs = []
                            for kb in range(t1 + 1):
                                if kb < t0:
                                    segs.append((kb, 0, 2 * P, None))
                                elif kb == t0:
                                    segs.append((kb, 0, 2 * P, 0))
                                else:
                                    segs.append((kb, P, P, 0))
                            # chunk segments into wide S tiles (<=512 cols)
                            chunks, cur, w = [], [], 0
                            for s in segs:
                                if w + s[2] > 4 * P:
                                    chunks.append(cur); cur, w = [], 0
                                cur.append(s); w += s[2]
                            if cur:
                                chunks.append(cur)
                            for chunk in chunks:
                                cw = sum(s[2] for s in chunk)
                                sp = spsp.tile([P, 4 * P], fp32, tag="sp")
                                off = 0
                                for (kb, qoff, w, mcol) in chunk:
                                    nc.tensor.matmul(
                                        sp[:, off:off + w],
                                        gkt_sb[:, 8 * r + hkv, kb * P:(kb + 1) * P],
                                        qt_sb[:, h, qs + qoff:qs + qoff + w],
                                        start=True, stop=True)
                                    off += w
                                pt = attwp.tile([P, 4 * P], bf16, tag="pt")
                                nc.scalar.activation(pt[:, :cw], sp[:, :cw], Exp)
                                off = 0
                                for (kb, qoff, w, mcol) in chunk:
                                    if mcol is not None:
                                        nc.vector.tensor_mul(
                                            pt[:, off + mcol:off + mcol + P],
                                            pt[:, off + mcol:off + mcol + P],
                                            mask_sb[:, r, :])
                                    off += w
                                off = 0
                                for (kb, qoff, w, mcol) in chunk:
                                    last = (r == 3 and kb == t1)
                                    nc.tensor.matmul(
                                        otp[:, qoff:qoff + w],
                                        gv_sb[:, 4 * r + kb, hkv * P:(hkv + 1) * P],
                                        pt[:, off:off + w], start=first, stop=last,
                                        skip_group_check=True)
                                    nc.tensor.matmul(
                                        den[:, qoff:qoff + w], ones_mat[:, :],
                                        pt[:, off:off + w], start=first, stop=last,
                                        skip_group_check=True)
                                    first = False
                                    off += w
                        # epilogue: normalize + vb, write bf16 to ot_sb
                        dinv = attep.tile([P, 2 * P], fp32, tag="dinv")
                        nc.vector.reciprocal(dinv[:, :], den[:, :])
                        tmpo = attep.tile([P, 2 * P], fp32, tag="tmpo")
                        nc.vector.tensor_mul(tmpo[:, :], otp[:, :], dinv[:, :])
                        nc.vector.tensor_scalar_add(
                            ot_sb[:, h, qs:qs + 2 * P], tmpo[:, :],
                            vb_sb[:, hkv:hkv + 1])

            # ================= output projection ===========================
            with tc.tile_pool(name="acc_o", bufs=1, space="PSUM") as oaccp:
                for p in range(4 if _PH == "full" else 0):
                    ops = [oaccp.tile([P, MLOC], fp32, tag=f"oacc{i}", name=f"ops{i}") for i in range(8)]
                    for k in range(KT):
                        owt = wsp.tile([P, 1024], bf16, tag="wt")
                        nc.sync.dma_start(
                            owt[:, :], ow[k * P:(k + 1) * P, p * 1024:(p + 1) * 1024])
                        for ff in range(8):
                            nc.tensor.matmul(
                                ops[ff][:, :], owt[:, ff * P:(ff + 1) * P],
                                ot_sb[:, k, :], start=(k == 0), stop=(k == KT - 1))
                    for ff in range(8):
                        o_st = stagep.tile([P, MLOC], fp32, tag="ostage")
                        nc.scalar.copy(o_st[:, :], ops[ff][:, :])
                        fo = p * 8 + ff
                        nc.sync.dma_start(out[fo * P:(fo + 1) * P, :], o_st[:, :])

    nc.compile()
    return nc


def _host_prep(x, freqs_cos, freqs_sin, qw, qb, kw, kb, vw, vb, ow):
    """Build per-core input maps (host-side sharding + layout prep)."""
    # pair permutation within each 128-wide head block: evens then odds
    pp = np.concatenate([np.arange(0, HD, 2), np.arange(1, HD, 2)])
    qperm = np.concatenate([h * HD + pp for h in range(NQH)])
    kperm = np.concatenate([h * HD + pp for h in range(NKVH)])
    scale = 1.0 / math.sqrt(HD)

    qw_p = np.ascontiguousarray((qw[:, qperm] * scale)).astype(BF16)
    qb_p = np.ascontiguousarray(
        (qb[qperm] * scale).reshape(NQH, HD).T.astype(np.float32))
    kw_p = np.ascontiguousarray(kw[:, kperm]).astype(BF16)
    kb_p = np.ascontiguousarray(kb[kperm].reshape(NKVH, HD).T.astype(np.float32))
    vb_p = np.ascontiguousarray(vb.reshape(NKVH, HD).T.astype(np.float32))
    vw_b = np.ascontiguousarray(vw).astype(BF16)
    ow_b = np.ascontiguousarray(ow).astype(BF16)

    in_maps = []
    for c in range(N_CORES):
        b, j = c // 4, c % 4
        idx = np.arange(j, S, 4)
        xt = np.ascontiguousarray(x[b][idx].T).astype(BF16)
        c = freqs_cos[idx].T.astype(np.float32)       # [64, MLOC]
        s = freqs_sin[idx].T.astype(np.float32)
        cosT = np.ascontiguousarray(np.vstack([c, c]))        # [128, MLOC]
        sinT = np.ascontiguousarray(np.vstack([-s, s]))

        tm = np.zeros((4 * P, P), np.float32)
        for r in range(4):
            tm[r * P:(r + 1) * P] = np.triu(np.ones((P, P), np.float32),
                                            0 if r <= j else 1)
        in_maps.append(dict(
            xt=xt, qw=qw_p, kw=kw_p, vw=vw_b, ow=ow_b,
            qb2=qb_p, kb2=kb_p, vb2=vb_p, cosT=cosT, sinT=sinT,
            trimask=tm.astype(BF16)))
    return in_maps


def kernel(x, freqs_cos, freqs_sin, qw, qb, kw, kb, vw, vb, ow, start_pos,
           _want_trace=False, _trace_kwargs=None):
    from concourse.bass_utils import run_bass_kernel_spmd

    if "nc" not in _GRAPH_CACHE:
        _GRAPH_CACHE["nc"] = _build_graph()
    nc = _GRAPH_CACHE["nc"]

    in_maps = _host_prep(np.asarray(x, np.float32), np.asarray(freqs_cos, np.float32),
                         np.asarray(freqs_sin, np.float32), np.asarray(qw, np.float32),
                         np.asarray(qb, np.float32), np.asarray(kw, np.float32),
                         np.asarray(kb, np.float32), np.asarray(vw, np.float32),
                         np.asarray(vb, np.float32), np.asarray(ow, np.float32))

    kw_ = dict(trace=True, **(_trace_kwargs or {})) if _want_trace else {}
    res = run_bass_kernel_spmd(nc, in_maps, core_ids=list(range(N_CORES)), **kw_)

    out = np.empty((B, S, D), np.float32)
    for c in range(N_CORES):
        b, j = c // 4, c % 4
        out[b, j::4, :] = res.results[c]["out"].T
    if _want_trace:
        return out, res
    return out

